# revision 10
# baseline (speedup 1.0000x reference)
"""Trainium2 Bass kernel for the MoE block (nn_MoEBlock_5592047420171).

Strategy: data-parallel over tokens across 8 NeuronCores (1024 tokens/core,
all weights replicated; no collectives).  Per core, layout A (d_ff on
partitions, tokens on the free dim):

  out[t,:] = v_t * sum_{e in top2(t)} relu(hs wi^T + bi + (hs lA_e^T) lB_e^T)
             @ wo^T + 2 v_t * bo

Key tricks:
  * router computes P=exp(logits) and uses the top-8 DVE sort: val_sum v and
    the top-2 membership mask come from (P >= second_max).
  * per-expert top-2 masking is additive pre-relu: s~_e = base + delta_e +
    (c_e - 1)*1e4, so unselected experts die in the relu.  The mask row rides
    the K=17 LoRA-delta matmul (ones row appended to lora_B^T).
  * base (+bi, via augmented contraction) is computed once per tile and added
    into each expert's PSUM bank with an identity matmul.
  * the shared val_sum v is pulled out of the expert sum; one broadcast
    multiply per tile.  2*v*bo rides the wo matmul as a K=1 rank-1 term.
  * big matmuls run as float32r (full-rate), router runs real fp32 so top-2
    selection matches the fp32 reference; H and the wo matmul are bf16.
"""

import numpy as np
from contextlib import ExitStack

import concourse.bass as bass
import concourse.tile as tile
from concourse import bacc, mybir
from concourse.bass_utils import run_bass_kernel_spmd
from concourse.masks import make_identity

B, S, DM, FF, E, RK = 4, 2048, 1024, 4096, 8, 16
NCORES = 8
TOK = B * S            # 8192 tokens
T = TOK // NCORES      # 1024 tokens per core
TCH = T // 128         # 8 token chunks of 128
FCH = FF // 128        # 32 d_ff chunks of 128
TT = 512               # token tile width (free dim of big matmuls)
NTT = T // TT          # 2 token tiles
NKC = 9                # contraction chunks: 8 x 128 d_model + 1 aug (bias)
BIG = 1.0e4

F32 = mybir.dt.float32
F32R = mybir.dt.float32r
BF16 = mybir.dt.bfloat16
AX = mybir.AxisListType
ALU = mybir.AluOpType
AF = mybir.ActivationFunctionType


def _r(ap):
    return ap.bitcast(F32R)


def build_bass():
    nc = bacc.Bacc("TRN2", target_bir_lowering=False)

    hs9 = nc.declare_dram_parameter("hs9", [128, NKC, T], F32R, isOutput=False)
    wi9 = nc.declare_dram_parameter("wi9", [FCH, 128, NKC, 128], F32R, isOutput=False)
    lAd = nc.declare_dram_parameter("lAd", [128, 8, 256], F32R, isOutput=False)
    LBd = nc.declare_dram_parameter("LBd", [FCH, 128, 2, 128], F32R, isOutput=False)
    gw9 = nc.declare_dram_parameter("gw9", [128, NKC, 8], F32, isOutput=False)
    woB = nc.declare_dram_parameter("woB", [FCH, 128, DM], BF16, isOutput=False)
    bo2 = nc.declare_dram_parameter("bo2", [1, DM], BF16, isOutput=False)
    outT = nc.declare_dram_parameter("outT", [DM, T], F32, isOutput=True)

    hs9, wi9, lAd, LBd, gw9, woB, bo2, outT = (
        h.ap() for h in (hs9, wi9, lAd, LBd, gw9, woB, bo2, outT))

    with tile.TileContext(nc) as tc, ExitStack() as ctx:
        persist = ctx.enter_context(tc.tile_pool(name="persist", bufs=1))
        dram = ctx.enter_context(tc.tile_pool(name="dram", bufs=1, space="DRAM"))

        # ---- resident tensors ----
        hs_sb = persist.tile([128, NKC, T], F32R, tag="hs")
        nc.sync.dma_start(out=hs_sb, in_=hs9)
        gw_sb = persist.tile([128, NKC, 8], F32, tag="gw")
        nc.sync.dma_start(out=gw_sb, in_=gw9)
        lA_sb = persist.tile([128, 8, 256], F32R, tag="lA")
        nc.sync.dma_start(out=lA_sb, in_=lAd)
        bo2_sb = persist.tile([1, DM], BF16, tag="bo2")
        nc.sync.dma_start(out=bo2_sb, in_=bo2)
        ident = persist.tile([128, 128], F32, tag="ident")
        make_identity(nc, ident)
        ident_r = persist.tile([128, 128], F32R, tag="identr")
        nc.scalar.copy(out=ident_r, in_=ident)
        ones8 = persist.tile([128, 8], F32, tag="ones8")
        nc.vector.memset(ones8, 1.0)

        mT_sb = persist.tile([8, T], F32R, tag="mT")       # (c-1)*BIG rows
        vT_sb = persist.tile([1, T], F32, tag="vT")       # val_sum row (f32)
        vTb_sb = persist.tile([1, T], BF16, tag="vTb")    # val_sum row (bf16)
        V_b = persist.tile([128, T], BF16, tag="Vb")      # val_sum bcast
        tA_sb = [
            [persist.tile([128, TT], F32R, tag=f"tA{tt}{x}", name=f"tA{tt}{x}") for x in range(2)]
            for tt in range(NTT)
        ]
        H_sb = [persist.tile([128, T], BF16, tag=f"H{fc}", name=f"H{fc}") for fc in range(FCH)]
        vrow = dram.tile([1, T], BF16, tag="vrow")

        # ---- phase 1: router ----
        with (
            tc.tile_pool(name="r_ps", bufs=2, space="PSUM") as r_ps,
            tc.tile_pool(name="tr_ps", bufs=2, space="PSUM") as tr_ps,
            tc.tile_pool(name="r_sb", bufs=3) as r_sb,
        ):
            for tch in range(TCH):
                tsl = slice(tch * 128, (tch + 1) * 128)
                lg = r_ps.tile([128, 8], F32, tag="lg")
                for ci in range(NKC):
                    nc.tensor.matmul(
                        lg,
                        lhsT=hs_sb[:, ci, tsl].bitcast(F32),
                        rhs=gw_sb[:, ci, :],
                        start=(ci == 0), stop=(ci == NKC - 1),
                    )
                P = r_sb.tile([128, 8], F32, tag="P")
                nc.scalar.activation(P, lg, AF.Exp)
                top8 = r_sb.tile([128, 8], F32, tag="top8")
                nc.vector.max(out=top8, in_=P)
                sP = r_sb.tile([128, 1], F32, tag="sP")
                nc.vector.tensor_reduce(out=sP, in_=P, axis=AX.X, op=ALU.add)
                rv = r_sb.tile([128, 1], F32, tag="rv")
                nc.vector.reciprocal(rv, sP)
                mb8 = r_sb.tile([128, 8], F32, tag="mb8")
                # mb8: ((P >= m2) - 1) * BIG ; vc: v = (m1+m2)*rv
                cm1 = r_sb.tile([128, 8], F32, tag="cm1")
                nc.vector.scalar_tensor_tensor(
                    out=cm1, in0=P, scalar=top8[:, 1:2], in1=ones8,
                    op0=ALU.is_ge, op1=ALU.subtract,
                )
                nc.vector.tensor_scalar(
                    out=mb8, in0=cm1, scalar1=float(BIG), scalar2=None,
                    op0=ALU.mult,
                )
                vc = r_sb.tile([128, 1], F32, tag="vc")
                nc.vector.scalar_tensor_tensor(
                    out=vc, in0=top8[:, 0:1], scalar=top8[:, 1:2],
                    in1=rv, op0=ALU.add, op1=ALU.mult,
                )
                trp = tr_ps.tile([8, 128], F32, tag="trp")
                nc.tensor.transpose(trp, mb8, ident)
                trpv = tr_ps.tile([1, 128], F32, tag="trpv")
                nc.tensor.transpose(trpv, vc, ident)
                nc.scalar.copy(out=mT_sb[:, tsl], in_=trp[0:8, :])
                nc.scalar.copy(out=vT_sb[:, tsl], in_=trpv[0:1, :])

        nc.scalar.copy(out=vTb_sb, in_=vT_sb)
        nc.sync.dma_start(out=vrow, in_=vTb_sb)
        nc.sync.dma_start(out=V_b, in_=vrow.to_broadcast([128, T]))

        # ---- phase 2: lora-A projections (tA), padded per 32-row groups ----
        with tc.tile_pool(name="tA_ps", bufs=2, space="PSUM") as tA_ps:
            for tt in range(NTT):
                tsl = slice(tt * TT, (tt + 1) * TT)
                for x in range(2):
                    pta = tA_ps.tile([128, TT], F32, tag="pta")
                    for ci in range(8):
                        nc.tensor.matmul(
                            pta,
                            lhsT=lA_sb[:, ci, x * 128:(x + 1) * 128],
                            rhs=hs_sb[:, ci, tsl],
                            start=(ci == 0), stop=(ci == 7),
                        )
                    nc.scalar.copy(out=tA_sb[tt][x], in_=pta)
                    # overwrite mask rows (32j+16) with the m_e rows
                    for j in range(4):
                        e = x * 4 + j
                        nc.sync.dma_start(
                            out=tA_sb[tt][x][32 * j + 16:32 * j + 17, :],
                            in_=mT_sb[e:e + 1, tsl],
                        )

        # ---- phase 3: main expert loop over (f-chunk, t-tile) ----
        with (
            tc.tile_pool(name="wi_sb", bufs=2) as wi_pool,
            tc.tile_pool(name="lb_sb", bufs=2) as lb_pool,
            tc.tile_pool(name="base_ps", bufs=2, space="PSUM") as base_pool,
            tc.tile_pool(name="bank_ps", bufs=6, space="PSUM") as bank_pool,
            tc.tile_pool(name="bs_sb", bufs=2) as bs_pool,
            tc.tile_pool(name="r_sb2", bufs=3) as rr_pool,
            tc.tile_pool(name="acc_sb", bufs=3) as acc_pool,
        ):
            for fc in range(FCH):
                wi_t = wi_pool.tile([128, NKC, 128], F32R, tag="wi")
                nc.sync.dma_start(out=wi_t, in_=wi9[fc])
                lb_t = lb_pool.tile([128, 2, 128], F32R, tag="lb")
                nc.sync.dma_start(out=lb_t, in_=LBd[fc])
                for tt in range(NTT):
                    tsl = slice(tt * TT, (tt + 1) * TT)
                    bps = base_pool.tile([128, TT], F32, tag="base")
                    for ci in range(NKC):
                        nc.tensor.matmul(
                            bps,
                            lhsT=wi_t[:, ci, :],
                            rhs=hs_sb[:, ci, tsl],
                            start=(ci == 0), stop=(ci == NKC - 1),
                        )
                    bs = bs_pool.tile([128, TT], F32R, tag="bs")
                    nc.scalar.copy(out=bs, in_=bps)
                    acc = None
                    for e in range(E):
                        x, j = e // 4, e % 4
                        bank = bank_pool.tile([128, TT], F32, tag="bank")
                        nc.tensor.matmul(
                            bank,
                            lhsT=lb_t[32 * j:32 * j + 17, x, :],
                            rhs=tA_sb[tt][x][32 * j:32 * j + 17, :],
                            start=True, stop=False,
                            tile_position=(32 * j, 0),
                        )
                        nc.tensor.matmul(
                            bank, lhsT=ident_r, rhs=bs,
                            start=False, stop=True,
                        )
                        r_t = rr_pool.tile([128, TT], BF16, tag="r")
                        nc.scalar.activation(r_t, bank, AF.Relu)
                        if acc is None:
                            acc = r_t
                        else:
                            nacc = acc_pool.tile([128, TT], BF16, tag="acc")
                            nc.vector.tensor_add(nacc, acc, r_t)
                            acc = nacc
                    nc.vector.tensor_mul(H_sb[fc][:, tsl], acc, V_b[:, tsl])

        # ---- phase 4: wo matmul (+ rank-1 2*v*bo term) ----
        with (
            tc.tile_pool(name="wo_sb", bufs=2) as wo_pool,
            tc.tile_pool(name="wo_ps", bufs=1, space="PSUM") as wo_ps,
            tc.tile_pool(name="o_sb", bufs=3) as o_pool,
        ):
            for tt in range(NTT):
                tsl = slice(tt * TT, (tt + 1) * TT)
                ops = [wo_ps.tile([128, TT], F32, tag=f"o{dc}", name=f"o{tt}_{dc}") for dc in range(8)]
                for fc in range(FCH):
                    wo_t = wo_pool.tile([128, DM], BF16, tag="wo")
                    nc.sync.dma_start(out=wo_t, in_=woB[fc])
                    for dc in range(8):
                        nc.tensor.matmul(
                            ops[dc],
                            lhsT=wo_t[:, dc * 128:(dc + 1) * 128],
                            rhs=H_sb[fc][:, tsl],
                            start=(fc == 0), stop=False,
                        )
                for dc in range(8):
                    nc.tensor.matmul(
                        ops[dc],
                        lhsT=bo2_sb[0:1, dc * 128:(dc + 1) * 128],
                        rhs=vTb_sb[0:1, tsl],
                        start=False, stop=True,
                    )
                    o_t = o_pool.tile([128, TT], F32, tag="ot")
                    nc.scalar.copy(out=o_t, in_=ops[dc])
                    nc.sync.dma_start(
                        out=outT[dc * 128:(dc + 1) * 128, tsl], in_=o_t
                    )

    nc.compile()
    return nc


def prep_inputs(hidden_states, wi, bi, wo, bo, lora_A, lora_B, gate_w, gate_b):
    """Host-side layout prep; returns per-core input maps."""
    import ml_dtypes
    bf = ml_dtypes.bfloat16
    f32 = np.float32
    hs = np.asarray(hidden_states, f32).reshape(TOK, DM)
    wi = np.asarray(wi, f32); bi = np.asarray(bi, f32)
    wo = np.asarray(wo, f32); bo = np.asarray(bo, f32)
    lora_A = np.asarray(lora_A, f32); lora_B = np.asarray(lora_B, f32)
    gate_w = np.asarray(gate_w, f32); gate_b = np.asarray(gate_b, f32)

    KD = NKC * 128  # 1152
    wiA = np.zeros((KD, FF), f32)
    wiA[:DM] = wi.T
    wiA[DM] = bi
    wi9 = np.ascontiguousarray(
        wiA.reshape(NKC, 128, FCH, 128).transpose(2, 1, 0, 3))

    lA_pad = np.zeros((DM, 256), f32)
    for e in range(E):
        x, j = e // 4, e % 4
        lA_pad[:, x * 128 + 32 * j: x * 128 + 32 * j + 16] = lora_A[e].T
    lAd = np.ascontiguousarray(lA_pad.reshape(8, 128, 256).transpose(1, 0, 2))

    LB_pad = np.zeros((2, 128, FF), f32)
    for e in range(E):
        x, j = e // 4, e % 4
        LB_pad[x, 32 * j: 32 * j + 16, :] = lora_B[e].T
        LB_pad[x, 32 * j + 16, :] = 1.0
    LBd = np.ascontiguousarray(
        LB_pad.reshape(2, 128, FCH, 128).transpose(2, 1, 0, 3))

    gwA = np.zeros((KD, E), f32)
    gwA[:DM] = gate_w.T
    gwA[DM] = gate_b
    gw9 = np.ascontiguousarray(gwA.reshape(NKC, 128, E).transpose(1, 0, 2))

    woB = np.ascontiguousarray(wo.T.reshape(FCH, 128, DM)).astype(bf)
    bo2 = (2.0 * bo).astype(bf).reshape(1, DM)

    shared = dict(wi9=wi9, lAd=lAd, LBd=LBd, gw9=gw9, woB=woB, bo2=bo2)
    in_maps = []
    for c in range(NCORES):
        hsA = np.zeros((KD, T), f32)
        hsA[:DM] = hs[c * T:(c + 1) * T].T
        hsA[DM] = 1.0
        hs9 = np.ascontiguousarray(hsA.reshape(NKC, 128, T).transpose(1, 0, 2))
        in_maps.append(dict(hs9=hs9, **shared))
    return in_maps


def run(in_maps, **kwargs):
    nc = build_bass()
    return nc, run_bass_kernel_spmd(nc, in_maps, list(range(NCORES)), **kwargs)


def kernel(hidden_states, wi, bi, wo, bo, lora_A, lora_B, gate_w, gate_b):
    in_maps = prep_inputs(hidden_states, wi, bi, wo, bo, lora_A, lora_B,
                          gate_w, gate_b)
    _, res = run(in_maps)
    out = np.stack([res.results[c]["outT"].T for c in range(NCORES)])
    return out.reshape(B, S, DM).astype(np.float32)


# revision 13
# speedup vs baseline: 2.5833x; 2.5833x over previous
"""Trainium2 Bass kernel for the MoE block (nn_MoEBlock_5592047420171).

Strategy: data-parallel over tokens across 8 NeuronCores (1024 tokens/core,
all weights replicated; no collectives).  Per core, layout A (d_ff on
partitions, tokens on the free dim):

  out[t,:] = v_t * (relu(base_t + delta_{e1(t),t}) + relu(base_t + delta_{e2(t),t}))
             @ wo^T + 2 v_t * bo
  base = hs wi^T + bi,  delta_e = (hs lA_e^T) lB_e^T,  v = top2 softmax mass

Key structure:
  * router computes P=exp(logits) in real fp32 (so top-2 selection matches the
    fp32 reference); the top-8 DVE sort gives max/second-max; one-hot masks of
    the first/second choice expert are built per token.
  * the 8 per-expert rank-16 lora paths collapse to TWO dense K=128 matmuls:
    tA (all experts' lora-A outputs, 8x16 rows) is masked per token by the
    first/second-choice one-hot (16-row groups), then multiplied by the
    concatenated lora-B.  No per-expert loop on the hot path.
  * base is computed once per tile and added into each choice's PSUM bank
    with an identity matmul; bi rides the PSUM->SBUF copy as an ACT bias.
  * val_sum v is pulled out of the expert sum (one broadcast multiply per
    tile); 2*v*bo rides the wo matmul as a K=1 rank-1 term.
  * all big matmuls are bf16 (fp32r measured at half the bf16 rate).
"""

import numpy as np
from contextlib import ExitStack

import concourse.bass as bass
import concourse.tile as tile
from concourse import bacc, mybir
from concourse.bass_utils import run_bass_kernel_spmd
from concourse.masks import make_identity

B, S, DM, FF, E, RK = 4, 2048, 1024, 4096, 8, 16
NCORES = 8
TOK = B * S            # 8192 tokens
T = TOK // NCORES      # 1024 tokens per core
TCH = T // 128         # 8 token chunks of 128
FCH = FF // 128        # 32 d_ff chunks of 128
TT = 512               # token tile width (free dim of big matmuls)
NTT = T // TT          # 2 token tiles
NKC = 9                # router contraction chunks: 8 x 128 d_model + bias
DCH = 8                # d_model chunks for the bf16 matmuls

F32 = mybir.dt.float32
BF16 = mybir.dt.bfloat16
AX = mybir.AxisListType
ALU = mybir.AluOpType
AF = mybir.ActivationFunctionType


def build_bass():
    nc = bacc.Bacc("TRN2", target_bir_lowering=False)

    hsR = nc.declare_dram_parameter("hsR", [128, NKC, T], F32, isOutput=False)
    hsB = nc.declare_dram_parameter("hsB", [128, DCH, T], BF16, isOutput=False)
    wiB = nc.declare_dram_parameter("wiB", [FCH, 128, DCH, 128], BF16, isOutput=False)
    biC = nc.declare_dram_parameter("biC", [128, FCH], F32, isOutput=False)
    lAc = nc.declare_dram_parameter("lAc", [128, DCH, 128], BF16, isOutput=False)
    lBc = nc.declare_dram_parameter("lBc", [FCH, 128, 128], BF16, isOutput=False)
    gw9 = nc.declare_dram_parameter("gw9", [128, NKC, 8], F32, isOutput=False)
    woB = nc.declare_dram_parameter("woB", [FCH, 128, DM], BF16, isOutput=False)
    bo2 = nc.declare_dram_parameter("bo2", [1, DM], BF16, isOutput=False)
    outT = nc.declare_dram_parameter("outT", [DM, T], F32, isOutput=True)

    hsR, hsB, wiB, biC, lAc, lBc, gw9, woB, bo2, outT = (
        h.ap() for h in (hsR, hsB, wiB, biC, lAc, lBc, gw9, woB, bo2, outT))

    with tile.TileContext(nc) as tc, ExitStack() as ctx:
        persist = ctx.enter_context(tc.tile_pool(name="persist", bufs=1))
        dram = ctx.enter_context(tc.tile_pool(name="dram", bufs=1, space="DRAM"))

        # ---- resident tensors ----
        hsR_sb = persist.tile([128, NKC, T], F32, tag="hsR")
        nc.sync.dma_start(out=hsR_sb, in_=hsR)
        hsB_sb = persist.tile([128, DCH, T], BF16, tag="hsB")
        nc.sync.dma_start(out=hsB_sb, in_=hsB)
        gw_sb = persist.tile([128, NKC, 8], F32, tag="gw")
        nc.sync.dma_start(out=gw_sb, in_=gw9)
        lA_sb = persist.tile([128, DCH, 128], BF16, tag="lA")
        nc.sync.dma_start(out=lA_sb, in_=lAc)
        bi_sb = persist.tile([128, FCH], F32, tag="bi")
        nc.sync.dma_start(out=bi_sb, in_=biC)
        bo2_sb = persist.tile([1, DM], BF16, tag="bo2")
        nc.sync.dma_start(out=bo2_sb, in_=bo2)
        ident = persist.tile([128, 128], F32, tag="ident")
        make_identity(nc, ident)
        identb = persist.tile([128, 128], BF16, tag="identb")
        nc.scalar.copy(out=identb, in_=ident)

        ohT_sb = persist.tile([16, T], BF16, tag="ohT")   # oh1 rows 0-7, oh2 8-15
        vT_sb = persist.tile([1, T], F32, tag="vT")       # val_sum row (f32)
        vTb_sb = persist.tile([1, T], BF16, tag="vTb")    # val_sum row (bf16)
        V_b = persist.tile([128, T], BF16, tag="Vb")      # val_sum bcast
        M1_sb = persist.tile([128, T], BF16, tag="M1")    # first-choice mask
        M2_sb = persist.tile([128, T], BF16, tag="M2")    # second-choice mask
        tA1_sb = persist.tile([128, T], BF16, tag="tA1")  # masked lora-A (1st)
        tA2_sb = persist.tile([128, T], BF16, tag="tA2")  # masked lora-A (2nd)
        H_sb = [persist.tile([128, T], BF16, tag=f"H{fc}", name=f"H{fc}")
                for fc in range(FCH)]
        vrow = dram.tile([1, T], BF16, tag="vrow")
        ohd = dram.tile([16, T], BF16, tag="ohd")

        # ---- phase 1: router ----
        with (
            tc.tile_pool(name="r_ps", bufs=2, space="PSUM") as r_ps,
            tc.tile_pool(name="tr_ps", bufs=2, space="PSUM") as tr_ps,
            tc.tile_pool(name="r_sb", bufs=3) as r_sb,
        ):
            for tch in range(TCH):
                tsl = slice(tch * 128, (tch + 1) * 128)
                lg = r_ps.tile([128, 8], F32, tag="lg")
                for ci in range(NKC):
                    nc.tensor.matmul(
                        lg,
                        lhsT=hsR_sb[:, ci, tsl],
                        rhs=gw_sb[:, ci, :],
                        start=(ci == 0), stop=(ci == NKC - 1),
                    )
                P = r_sb.tile([128, 8], F32, tag="P")
                nc.scalar.activation(P, lg, AF.Exp)
                top8 = r_sb.tile([128, 8], F32, tag="top8")
                nc.vector.max(out=top8, in_=P)
                sP = r_sb.tile([128, 1], F32, tag="sP")
                nc.vector.tensor_reduce(out=sP, in_=P, axis=AX.X, op=ALU.add)
                rv = r_sb.tile([128, 1], F32, tag="rv")
                nc.vector.reciprocal(rv, sP)
                # oh1 = (P == max); oh2 = (P >= m2) - oh1 ; v = (m1+m2)*rv
                ohb = r_sb.tile([128, 16], F32, tag="ohb")
                nc.vector.tensor_scalar(
                    out=ohb[:, 0:8], in0=P, scalar1=top8[:, 0:1], scalar2=None,
                    op0=ALU.is_equal,
                )
                nc.vector.scalar_tensor_tensor(
                    out=ohb[:, 8:16], in0=P, scalar=top8[:, 1:2],
                    in1=ohb[:, 0:8], op0=ALU.is_ge, op1=ALU.subtract,
                )
                vc = r_sb.tile([128, 1], F32, tag="vc")
                nc.vector.scalar_tensor_tensor(
                    out=vc, in0=top8[:, 0:1], scalar=top8[:, 1:2],
                    in1=rv, op0=ALU.add, op1=ALU.mult,
                )
                trp = tr_ps.tile([16, 128], F32, tag="trp")
                nc.tensor.transpose(trp, ohb, ident)
                trpv = tr_ps.tile([1, 128], F32, tag="trpv")
                nc.tensor.transpose(trpv, vc, ident)
                nc.scalar.copy(out=ohT_sb[:, tsl], in_=trp)
                nc.scalar.copy(out=vT_sb[:, tsl], in_=trpv[0:1, :])

        nc.scalar.copy(out=vTb_sb, in_=vT_sb)
        nc.sync.dma_start(out=vrow, in_=vTb_sb)
        nc.sync.dma_start(out=V_b, in_=vrow.to_broadcast([128, T]))
        nc.sync.dma_start(out=ohd, in_=ohT_sb)
        # expand one-hot rows to 16-row groups: M[16e+k, t] = oh[e, t]
        m1_src = bass.AP(tensor=ohd.tensor, offset=ohd.offset,
                         ap=[[T, 8], [0, 16], [1, T]])
        nc.sync.dma_start(out=M1_sb, in_=m1_src)
        m2_src = bass.AP(tensor=ohd.tensor, offset=ohd.offset + 8 * T,
                         ap=[[T, 8], [0, 16], [1, T]])
        nc.sync.dma_start(out=M2_sb, in_=m2_src)

        # ---- phase 2: lora-A projections + per-token choice masking ----
        with (
            tc.tile_pool(name="tA_ps", bufs=2, space="PSUM") as tA_ps,
            tc.tile_pool(name="tA_tmp", bufs=2) as tA_tmp,
        ):
            for tt in range(NTT):
                tsl = slice(tt * TT, (tt + 1) * TT)
                pta = tA_ps.tile([128, TT], F32, tag="pta")
                for ci in range(DCH):
                    nc.tensor.matmul(
                        pta,
                        lhsT=lA_sb[:, ci, :],
                        rhs=hsB_sb[:, ci, tsl],
                        start=(ci == 0), stop=(ci == DCH - 1),
                    )
                tAf = tA_tmp.tile([128, TT], BF16, tag="tAf")
                nc.scalar.copy(out=tAf, in_=pta)
                nc.vector.tensor_tensor(
                    out=tA1_sb[:, tsl], in0=tAf, in1=M1_sb[:, tsl], op=ALU.mult)
                nc.vector.tensor_tensor(
                    out=tA2_sb[:, tsl], in0=tAf, in1=M2_sb[:, tsl], op=ALU.mult)

        # ---- phase 3: main loop over (f-chunk, t-tile) ----
        with (
            tc.tile_pool(name="wi_sb", bufs=2) as wi_pool,
            tc.tile_pool(name="lb_sb", bufs=2) as lb_pool,
            tc.tile_pool(name="base_ps", bufs=2, space="PSUM") as base_pool,
            tc.tile_pool(name="bank_ps", bufs=4, space="PSUM") as bank_pool,
            tc.tile_pool(name="bs_sb", bufs=2) as bs_pool,
            tc.tile_pool(name="r_sb2", bufs=4) as rr_pool,
            tc.tile_pool(name="acc_sb", bufs=2) as acc_pool,
        ):
            for fc in range(FCH):
                wi_t = wi_pool.tile([128, DCH, 128], BF16, tag="wi")
                nc.sync.dma_start(out=wi_t, in_=wiB[fc])
                lb_t = lb_pool.tile([128, 128], BF16, tag="lb")
                nc.sync.dma_start(out=lb_t, in_=lBc[fc])
                for tt in range(NTT):
                    tsl = slice(tt * TT, (tt + 1) * TT)
                    bps = base_pool.tile([128, TT], F32, tag="base")
                    for ci in range(DCH):
                        nc.tensor.matmul(
                            bps,
                            lhsT=wi_t[:, ci, :],
                            rhs=hsB_sb[:, ci, tsl],
                            start=(ci == 0), stop=(ci == DCH - 1),
                        )
                    bs = bs_pool.tile([128, TT], BF16, tag="bs")
                    nc.vector.tensor_scalar(
                        out=bs, in0=bps, scalar1=bi_sb[:, fc:fc + 1],
                        scalar2=None, op0=ALU.add)
                    rts = []
                    for tA_m in (tA1_sb, tA2_sb):
                        bank = bank_pool.tile([128, TT], F32, tag="bank")
                        nc.tensor.matmul(
                            bank, lhsT=lb_t, rhs=tA_m[:, tsl],
                            start=True, stop=False,
                        )
                        nc.tensor.matmul(
                            bank, lhsT=identb, rhs=bs,
                            start=False, stop=True,
                        )
                        r_t = rr_pool.tile([128, TT], BF16, tag="r")
                        nc.scalar.activation(r_t, bank, AF.Relu)
                        rts.append(r_t)
                    acc = acc_pool.tile([128, TT], BF16, tag="acc")
                    nc.vector.tensor_tensor(
                        out=acc, in0=rts[0], in1=rts[1], op=ALU.add)
                    nc.vector.tensor_tensor(
                        out=H_sb[fc][:, tsl], in0=acc, in1=V_b[:, tsl],
                        op=ALU.mult)

        # ---- phase 4: wo matmul (+ rank-1 2*v*bo term) ----
        with (
            tc.tile_pool(name="wo_sb", bufs=2) as wo_pool,
            tc.tile_pool(name="wo_ps", bufs=1, space="PSUM") as wo_ps,
            tc.tile_pool(name="o_sb", bufs=3) as o_pool,
        ):
            for tt in range(NTT):
                tsl = slice(tt * TT, (tt + 1) * TT)
                ops = [wo_ps.tile([128, TT], F32, tag=f"o{dc}", name=f"o{tt}_{dc}")
                       for dc in range(8)]
                for fc in range(FCH):
                    wo_t = wo_pool.tile([128, DM], BF16, tag="wo")
                    nc.sync.dma_start(out=wo_t, in_=woB[fc])
                    for dc in range(8):
                        nc.tensor.matmul(
                            ops[dc],
                            lhsT=wo_t[:, dc * 128:(dc + 1) * 128],
                            rhs=H_sb[fc][:, tsl],
                            start=(fc == 0), stop=False,
                        )
                for dc in range(8):
                    nc.tensor.matmul(
                        ops[dc],
                        lhsT=bo2_sb[0:1, dc * 128:(dc + 1) * 128],
                        rhs=vTb_sb[0:1, tsl],
                        start=False, stop=True,
                    )
                    o_t = o_pool.tile([128, TT], F32, tag="ot")
                    nc.scalar.copy(out=o_t, in_=ops[dc])
                    nc.sync.dma_start(
                        out=outT[dc * 128:(dc + 1) * 128, tsl], in_=o_t
                    )

    nc.compile()
    return nc


def prep_inputs(hidden_states, wi, bi, wo, bo, lora_A, lora_B, gate_w, gate_b):
    """Host-side layout prep; returns per-core input maps."""
    import ml_dtypes
    bf = ml_dtypes.bfloat16
    f32 = np.float32
    hs = np.asarray(hidden_states, f32).reshape(TOK, DM)
    wi = np.asarray(wi, f32); bi = np.asarray(bi, f32)
    wo = np.asarray(wo, f32); bo = np.asarray(bo, f32)
    lora_A = np.asarray(lora_A, f32); lora_B = np.asarray(lora_B, f32)
    gate_w = np.asarray(gate_w, f32); gate_b = np.asarray(gate_b, f32)

    # wi^T in (fc, d, ci, f) bf16 chunks
    wiB = np.ascontiguousarray(
        wi.T.reshape(DCH, 128, FCH, 128).transpose(2, 1, 0, 3)).astype(bf)
    biC = np.ascontiguousarray(bi.reshape(FCH, 128).T)

    # lora-A concatenated: columns 16e+r = lora_A[e,r,:]
    lA_cat = np.concatenate([lora_A[e].T for e in range(E)], axis=1)  # [DM,128]
    lAc = np.ascontiguousarray(lA_cat.reshape(DCH, 128, 128).transpose(1, 0, 2)
                               ).astype(bf)
    # lora-B concatenated: rows 16e+r = lora_B[e,:,r]
    lB_cat = np.concatenate([lora_B[e].T for e in range(E)], axis=0)  # [128,FF]
    lBc = np.ascontiguousarray(
        lB_cat.reshape(128, FCH, 128).transpose(1, 0, 2)).astype(bf)

    KD = NKC * 128
    gwA = np.zeros((KD, E), f32)
    gwA[:DM] = gate_w.T
    gwA[DM] = gate_b
    gw9 = np.ascontiguousarray(gwA.reshape(NKC, 128, E).transpose(1, 0, 2))

    woB = np.ascontiguousarray(wo.T.reshape(FCH, 128, DM)).astype(bf)
    bo2 = (2.0 * bo).astype(bf).reshape(1, DM)

    shared = dict(wiB=wiB, biC=biC, lAc=lAc, lBc=lBc, gw9=gw9, woB=woB, bo2=bo2)
    in_maps = []
    for c in range(NCORES):
        hsc = hs[c * T:(c + 1) * T]
        hsA = np.zeros((KD, T), f32)
        hsA[:DM] = hsc.T
        hsA[DM] = 1.0
        hsR = np.ascontiguousarray(hsA.reshape(NKC, 128, T).transpose(1, 0, 2))
        hsB = np.ascontiguousarray(
            hsc.T.reshape(DCH, 128, T).transpose(1, 0, 2)).astype(bf)
        in_maps.append(dict(hsR=hsR, hsB=hsB, **shared))
    return in_maps


def run(in_maps, **kwargs):
    nc = build_bass()
    return nc, run_bass_kernel_spmd(nc, in_maps, list(range(NCORES)), **kwargs)


def kernel(hidden_states, wi, bi, wo, bo, lora_A, lora_B, gate_w, gate_b):
    in_maps = prep_inputs(hidden_states, wi, bi, wo, bo, lora_A, lora_B,
                          gate_w, gate_b)
    _, res = run(in_maps)
    out = np.stack([res.results[c]["outT"].T for c in range(NCORES)])
    return out.reshape(B, S, DM).astype(np.float32)
